# revision 1
# baseline (speedup 1.0000x reference)
"""Fused 7-gate continuous-time LSTM cell on 8 Trainium2 NeuronCores.

Data-parallel over the batch dim: each core gets B/8 = 1024 rows, the
fused gate weight W [2048, 7*2048] is replicated. Per core:
  g = hx @ W + b   (fp32r matmuls, K accumulated in PSUM)
  gates -> sigmoid/tanh/softplus, then the continuous-time cell update.
"""

import sys

sys.path.insert(0, "/opt/trn_rl_repo")

import numpy as np

import concourse.bass as bass
import concourse.mybir as mybir
import concourse.tile as tile
from concourse import bacc, bass_utils
from concourse.masks import make_identity

B, D, H, NG = 8192, 2048, 2048, 7
N_CORES = 8
BL = B // N_CORES  # 1024 rows per core
P = 128
HB = 256  # H-column block per matmul (fp32 PSUM: <=512)
N_HB = H // HB  # 8
KT = D // P  # 16 contraction tiles
MT = BL // P  # 8 batch tiles per core

F32 = mybir.dt.float32
MM_DT = mybir.dt.float32r  # PE runs fp32r at 1 cyc/row (vs 4 for fp32)

AF = mybir.ActivationFunctionType
# i1,i2,f1,f2,o -> Sigmoid, z -> Tanh, d -> softplus via Ln(1+Exp(x))
# (no ACT table set holds sigmoid+tanh+softplus+exp together; Sigmoid/Tanh
# live in one set and Exp/Ln in another, so compose softplus from Exp+Ln)
GATE_FUNC = [AF.Sigmoid] * 5 + [AF.Tanh, None]

_cached_nc = None


def _build():
    nc = bacc.Bacc("TRN2", target_bir_lowering=False, debug=False,
                   num_devices=N_CORES)
    hx = nc.dram_tensor("hx", [BL, D], F32, kind="ExternalInput").ap()
    cx1 = nc.dram_tensor("cx1", [BL, H], F32, kind="ExternalInput").ap()
    cx2 = nc.dram_tensor("cx2", [BL, H], F32, kind="ExternalInput").ap()
    tj = nc.dram_tensor("tj", [BL, 1], F32, kind="ExternalInput").ap()
    dt_in = nc.dram_tensor("dt", [BL, 1], F32, kind="ExternalInput").ap()
    W = nc.dram_tensor("W", [D, NG * H], F32, kind="ExternalInput").ap()
    b = nc.dram_tensor("b", [NG, H], F32, kind="ExternalInput").ap()
    out = nc.dram_tensor("out", [3, BL, H], F32, kind="ExternalOutput").ap()

    from contextlib import ExitStack

    with tile.TileContext(nc) as tc, ExitStack() as ctx:
        const_pool = ctx.enter_context(tc.tile_pool(name="const", bufs=1))
        psum_pool = ctx.enter_context(tc.tile_pool(name="ps", bufs=8, space="PSUM"))
        small_pool = ctx.enter_context(tc.tile_pool(name="small", bufs=4))

        ident = const_pool.tile([P, P], F32)
        make_identity(nc, ident)

        # hx transposed: [d-partition, k-tile, b-col] resident all kernel (8MB)
        hxT = const_pool.tile([P, KT, BL], MM_DT)
        # -u per batch row, u = (tj+dt)-tj, laid out [128, m-tile]
        negu = const_pool.tile([P, MT], F32)

        for m in range(MT):
            ms = slice(m * P, (m + 1) * P)
            tjt = small_pool.tile([P, 1], F32, tag="tj")
            dtt = small_pool.tile([P, 1], F32, tag="dt")
            nc.sync.dma_start(tjt, tj[ms, :])
            nc.sync.dma_start(dtt, dt_in[ms, :])
            tsum = small_pool.tile([P, 1], F32, tag="ts")
            nc.vector.tensor_add(tsum, tjt, dtt)
            u = small_pool.tile([P, 1], F32, tag="u")
            nc.vector.tensor_sub(u, tsum, tjt)
            nc.vector.tensor_scalar_mul(negu[:, m : m + 1], u, -1.0)

        # phase 1: load hx row-tiles and PE-transpose into hxT
        with tc.tile_pool(name="stag", bufs=3) as stag:
            for m in range(MT):
                hxm = stag.tile([P, D], F32, tag="hxm")
                nc.gpsimd.dma_start(hxm, hx[m * P : (m + 1) * P, :])
                for k in range(KT):
                    pst = psum_pool.tile([P, P], F32, tag="ps")
                    nc.tensor.transpose(pst, hxm[:, k * P : (k + 1) * P], ident)
                    nc.vector.tensor_copy(
                        out=hxT[:, k, m * P : (m + 1) * P], in_=pst
                    )

        wpool = ctx.enter_context(tc.tile_pool(name="w", bufs=16))
        bpool = ctx.enter_context(tc.tile_pool(name="bb", bufs=1))
        gates_pool = ctx.enter_context(tc.tile_pool(name="gates", bufs=NG + 1))
        cx_pool = ctx.enter_context(tc.tile_pool(name="cx", bufs=16))
        tmp_pool = ctx.enter_context(tc.tile_pool(name="tmp", bufs=2))
        out_pool = ctx.enter_context(tc.tile_pool(name="outp", bufs=3))

        for hb in range(N_HB):
            cs = slice(hb * HB, (hb + 1) * HB)
            # bias block for all 7 gates, broadcast to 128 partitions
            bsl = b[:, cs]  # [NG, HB]
            b_bcast = bass.AP(
                tensor=bsl.tensor, offset=bsl.offset, ap=[[0, P], *bsl.ap]
            )
            bt = bpool.tile([P, NG, HB], F32, tag="bt")
            nc.gpsimd.dma_start(bt, b_bcast)

            cx1ts, cx2ts = [], []
            for m in range(MT):
                ms = slice(m * P, (m + 1) * P)
                cx1t = cx_pool.tile([P, HB], F32, tag="cx1", name=f"cx1_{hb}_{m}")
                nc.gpsimd.dma_start(cx1t, cx1[ms, cs])
                cx1ts.append(cx1t)
                cx2t = cx_pool.tile([P, HB], F32, tag="cx2", name=f"cx2_{hb}_{m}")
                nc.gpsimd.dma_start(cx2t, cx2[ms, cs])
                cx2ts.append(cx2t)

            gates = []
            for g in range(NG):
                gt = gates_pool.tile([P, MT, HB], F32, tag="gates")
                gates.append(gt)
                ps = [
                    psum_pool.tile([P, HB], F32, tag="ps", name=f"ps_{hb}_{g}_{m}")
                    for m in range(MT)
                ]
                for k in range(KT):
                    wt = wpool.tile([P, HB], MM_DT, tag="w")
                    nc.sync.dma_start(
                        wt,
                        W[
                            k * P : (k + 1) * P,
                            g * H + hb * HB : g * H + hb * HB + HB,
                        ].bitcast(MM_DT),
                    )
                    for m in range(MT):
                        nc.tensor.matmul(
                            ps[m][:],
                            hxT[:, k, m * P : (m + 1) * P],
                            wt[:],
                            start=(k == 0),
                            stop=(k == KT - 1),
                        )
                for m in range(MT):
                    tmp = tmp_pool.tile([P, HB], F32, tag="ba", bufs=4)
                    nc.vector.tensor_add(tmp, ps[m][:], bt[:, g, :])
                    if GATE_FUNC[g] is not None:
                        nc.scalar.activation(gt[:, m, :], tmp, GATE_FUNC[g])
                    else:
                        # softplus(x) = ln(1 + exp(x)); x <= ~6 so no overflow
                        ex = tmp_pool.tile([P, HB], F32, tag="tt", bufs=6)
                        nc.scalar.activation(ex, tmp, AF.Exp)
                        nc.scalar.activation(gt[:, m, :], ex, AF.Ln, bias=1.0)

            i1, i2, f1, f2, o, z, dc = gates
            for m in range(MT):
                ms = slice(m * P, (m + 1) * P)
                cx1t = cx1ts[m]
                cx2t = cx2ts[m]

                t1 = tmp_pool.tile([P, HB], F32, tag="tt", bufs=6)
                nc.vector.tensor_mul(t1, f1[:, m, :], cx1t)
                t2 = tmp_pool.tile([P, HB], F32, tag="tt", bufs=6)
                nc.vector.tensor_mul(t2, i1[:, m, :], z[:, m, :])
                cy1 = out_pool.tile([P, HB], F32, tag="cy1")
                nc.vector.tensor_add(cy1, t1, t2)

                t3 = tmp_pool.tile([P, HB], F32, tag="tt", bufs=6)
                nc.vector.tensor_mul(t3, f2[:, m, :], cx2t)
                t4 = tmp_pool.tile([P, HB], F32, tag="tt", bufs=6)
                nc.vector.tensor_mul(t4, i2[:, m, :], z[:, m, :])
                cy2 = out_pool.tile([P, HB], F32, tag="cy2")
                nc.vector.tensor_add(cy2, t3, t4)

                # E = exp(-decay * u)
                E = tmp_pool.tile([P, HB], F32, tag="tt", bufs=6)
                nc.scalar.activation(E, dc[:, m, :], AF.Exp,
                                     scale=negu[:, m : m + 1])
                dif = tmp_pool.tile([P, HB], F32, tag="tt", bufs=6)
                nc.vector.tensor_sub(dif, cy1, cy2)
                t5 = tmp_pool.tile([P, HB], F32, tag="tt", bufs=6)
                nc.vector.tensor_mul(t5, dif, E)
                ct = tmp_pool.tile([P, HB], F32, tag="tt", bufs=6)
                nc.vector.tensor_add(ct, cy2, t5)
                tct = tmp_pool.tile([P, HB], F32, tag="tt", bufs=6)
                nc.scalar.activation(tct, ct, AF.Tanh)
                ht = out_pool.tile([P, HB], F32, tag="ht")
                nc.vector.tensor_mul(ht, o[:, m, :], tct)

                nc.gpsimd.dma_start(out[0, ms, cs], cy1)
                nc.gpsimd.dma_start(out[1, ms, cs], cy2)
                nc.gpsimd.dma_start(out[2, ms, cs], ht)

    nc.compile()
    return nc


def _get_nc():
    global _cached_nc
    if _cached_nc is None:
        _cached_nc = _build()
    return _cached_nc


def kernel(hx, cx1, cx2, tj, dt, W, b, trace=False):
    nc = _get_nc()
    Wc = np.ascontiguousarray(W, dtype=np.float32)
    b2 = np.ascontiguousarray(b, dtype=np.float32).reshape(NG, H)
    in_maps = []
    for c in range(N_CORES):
        rs = slice(c * BL, (c + 1) * BL)
        in_maps.append(
            {
                "hx": np.ascontiguousarray(hx[rs], dtype=np.float32),
                "cx1": np.ascontiguousarray(cx1[rs], dtype=np.float32),
                "cx2": np.ascontiguousarray(cx2[rs], dtype=np.float32),
                "tj": np.ascontiguousarray(tj[rs], dtype=np.float32),
                "dt": np.ascontiguousarray(dt[rs], dtype=np.float32),
                "W": Wc,
                "b": b2,
            }
        )
    res = bass_utils.run_bass_kernel_spmd(
        nc, in_maps, core_ids=list(range(N_CORES)), trace=trace
    )
    out = np.concatenate([r["out"] for r in res.results], axis=1)
    if trace:
        kernel.last_exec_time_ns = res.exec_time_ns
        kernel.last_results = res
    return out

